# revision 5
# baseline (speedup 1.0000x reference)
"""Trainium2 Bass kernel for a SAGAN-style self-attention block.

Reference computation (per batch b):
    xc = x_ccd[b] reshaped [C, N]; xd = x_dem[b] reshaped [C, N]
    q  = (Wq @ xc).T + bq          # [N, 32]
    k  = Wk @ xd + bk              # [32, N]
    e  = q @ k                     # [N, N]
    a  = softmax(e, axis=-1)
    v  = Wv @ xd + bv              # [C, N]
    y  = gamma * (v @ a.T) + x_ccd[b]

Sharding: 8 cores = 4 batches x 2 query-row halves. Each core computes the
full k/v projections for its batch and a 2048-row slice of the attention
output. No collectives needed.

Per-core layout (all "transposed" so softmax's reduction lands on the PE
via an appended ones-column, avoiding partition-axis reductions):
    qt   [32, 2048]   (o on partitions)
    k    [32, 4096]
    vt   [128, 32, 257] bf16  (m on partitions, 32 m-chunks; col 256 = 1.0)
    eT   [m 128, n 512] tiles = k_chunk.T @ qt  -> exp on ACT -> bf16
    outU [n 128, 257]  = sum_m expT_chunk.T @ vt_chunk   (col 256 = denom)
    y    = transpose(outU[:, :256] * gamma/denom) + xc
"""

import numpy as np

import concourse.bacc as bacc
import concourse.mybir as mybir
import concourse.tile as tile
from concourse import bass
from concourse.bass_utils import run_bass_kernel_spmd

B, C, H, W = 4, 256, 64, 64
N = H * W  # 4096
NH = N // 2  # 2048 query rows per core
C8 = 32
P = 128
N_CORES = 8

FP32 = mybir.dt.float32
FP32R = mybir.dt.float32r
BF16 = mybir.dt.bfloat16

ts = bass.ts


def emit_body(nc, tc, t, pools):
    """Emit one iteration of the attention computation.

    t: dict of DRAM parameter handles. pools: dict of tile pools.
    """
    cpool = pools["const"]
    iopool = pools["io"]
    qkvpool = pools["qkv"]
    epool = pools["expp"]
    wpool = pools["work"]

    # ---- constants / weights -------------------------------------------------
    wqt = cpool.tile([P, 2, C8], FP32R, tag="wqt")
    nc.sync.dma_start(wqt[:], t["wqt"][:].rearrange("(j p) o -> p j o", p=P))
    wkt = cpool.tile([P, 2, C8], FP32R, tag="wkt")
    nc.sync.dma_start(wkt[:], t["wkt"][:].rearrange("(j p) o -> p j o", p=P))
    wvt = cpool.tile([P, 2, C], FP32R, tag="wvt")
    nc.sync.dma_start(wvt[:], t["wvt"][:].rearrange("(j p) o -> p j o", p=P))
    bq = cpool.tile([C8, 1], FP32, tag="bq")
    nc.sync.dma_start(bq[:], t["bq"][:])
    bk = cpool.tile([C8, 1], FP32, tag="bk")
    nc.sync.dma_start(bk[:], t["bk"][:])
    bvb = cpool.tile([P, C], FP32, tag="bvb")
    nc.sync.dma_start(bvb[:], t["bvb"][:])
    gam = cpool.tile([P, 1], FP32, tag="gam")
    nc.sync.dma_start(gam[:], t["gam"][:])
    ident = cpool.tile([P, P], FP32, tag="ident")
    nc.sync.dma_start(ident[:], t["ident"][:])

    # ---- activations (split DMAs so they spread across queues) --------------
    xd = iopool.tile([P, 2, N], FP32R, tag="xd")
    xd_r = t["xd"][:].rearrange("(j p) n -> p j n", p=P)
    for j in range(2):
        for s in range(4):
            nc.sync.dma_start(xd[:, j, ts(s, 1024)], xd_r[:, j, ts(s, 1024)])
    xc = iopool.tile([P, 2, NH], FP32, tag="xc")
    xc_r = t["xc"][:].rearrange("(j p) n -> p j n", p=P)
    for j in range(2):
        for s in range(2):
            nc.sync.dma_start(xc[:, j, ts(s, 1024)], xc_r[:, j, ts(s, 1024)])
    y_sb = iopool.tile([P, 2, NH], FP32, tag="y")

    # rounded copy of xc for the fp32r q-projection (xc itself stays exact
    # fp32 for the residual add)
    xcr = qkvpool.tile([P, 2, NH], FP32R, tag="xcr")
    for j in range(2):
        nc.vector.tensor_copy(xcr[:, j, :], xc[:, j, :])

    qt = qkvpool.tile([C8, NH], FP32R, tag="qt")
    k_sb = qkvpool.tile([C8, N], FP32R, tag="k")
    vt = qkvpool.tile([P, 32, C + 1], BF16, tag="vt")

    # ---- projections ---------------------------------------------------------
    with (
        tc.tile_pool(name="ps_p", bufs=2, space="PSUM") as ps_p,
        tc.tile_pool(name="ps_v", bufs=2, space="PSUM") as ps_v,
    ):
        for j in range(N // 512):  # k = Wk @ xd + bk
            pk = ps_p.tile([C8, 512], FP32, tag="pqk")
            nc.tensor.matmul(
                pk[:],
                wkt[:, 0, :],
                xd[:, 0, ts(j, 512)],
                start=True,
                stop=False,
            )
            nc.tensor.matmul(
                pk[:],
                wkt[:, 1, :],
                xd[:, 1, ts(j, 512)],
                start=False,
                stop=True,
            )
            nc.vector.tensor_scalar_add(k_sb[:, ts(j, 512)], pk[:], bk[:])
        for j in range(NH // 512):  # qt = Wq @ xc + bq
            pq = ps_p.tile([C8, 512], FP32, tag="pqk")
            nc.tensor.matmul(
                pq[:],
                wqt[:, 0, :],
                xcr[:, 0, ts(j, 512)],
                start=True,
                stop=False,
            )
            nc.tensor.matmul(
                pq[:],
                wqt[:, 1, :],
                xcr[:, 1, ts(j, 512)],
                start=False,
                stop=True,
            )
            nc.vector.tensor_scalar_add(qt[:, ts(j, 512)], pq[:], bq[:])
        for mi in range(32):  # vt = (Wv @ xd + bv).T, ones col appended
            pv = ps_v.tile([P, C], FP32, tag="pv")
            nc.tensor.matmul(
                pv[:],
                xd[:, 0, ts(mi, 128)],
                wvt[:, 0, :],
                start=True,
                stop=False,
            )
            nc.tensor.matmul(
                pv[:],
                xd[:, 1, ts(mi, 128)],
                wvt[:, 1, :],
                start=False,
                stop=True,
            )
            nc.vector.tensor_add(vt[:, mi, 0:C], pv[:], bvb[:])
            nc.vector.memset(vt[:, mi, C : C + 1], 1.0)

    # ---- attention -----------------------------------------------------------
    with (
        tc.tile_pool(name="ps_e", bufs=3, space="PSUM") as ps_e,
        tc.tile_pool(name="ps_u", bufs=2, space="PSUM") as ps_u,
        tc.tile_pool(name="ps_t", bufs=2, space="PSUM") as ps_t,
    ):
        for nch in range(NH // 512):
            ex = epool.tile([P, 32, 512], BF16, tag="expT")
            for mi in range(32):
                pe = ps_e.tile([P, 512], FP32, tag="en")
                nc.tensor.matmul(
                    pe[:],
                    k_sb[:, ts(mi, 128)],
                    qt[:, ts(nch, 512)],
                    start=True,
                    stop=True,
                )
                nc.scalar.activation(
                    ex[:, mi, :], pe[:], mybir.ActivationFunctionType.Exp
                )
            for ns in range(4):
                pu = ps_u.tile([P, C + 1], FP32, tag="outu")
                for mi in range(32):
                    nc.tensor.matmul(
                        pu[:],
                        ex[:, mi, ts(ns, 128)],
                        vt[:, mi, :],
                        start=(mi == 0),
                        stop=(mi == 31),
                    )
                recip = wpool.tile([P, 1], FP32, tag="recip")
                nc.vector.reciprocal(recip[:], pu[:, C : C + 1])
                scale = wpool.tile([P, 1], FP32, tag="scale")
                nc.vector.tensor_mul(scale[:], recip[:], gam[:])
                norm = wpool.tile([P, C], FP32, tag="norm")
                nc.vector.tensor_scalar_mul(norm[:], pu[:, 0:C], scale[:])
                ng = nch * 4 + ns
                for oc in range(2):
                    pt = ps_t.tile([P, P], FP32, tag="tr")
                    nc.tensor.transpose(pt[:], norm[:, ts(oc, 128)], ident[:])
                    nc.vector.tensor_add(
                        y_sb[:, oc, ts(ng, 128)], pt[:], xc[:, oc, ts(ng, 128)]
                    )

    # ---- store ---------------------------------------------------------------
    y_r = t["y"][:].rearrange("(j p) n -> p j n", p=P)
    for j in range(2):
        for s in range(2):
            nc.sync.dma_start(y_r[:, j, ts(s, 1024)], y_sb[:, j, ts(s, 1024)])


def build_nc(loop_reps=1):
    nc = bacc.Bacc("TRN2", target_bir_lowering=False, debug=False, num_devices=N_CORES)
    t = {
        "xc": nc.declare_dram_parameter("xc", [C, NH], FP32, isOutput=False),
        "xd": nc.declare_dram_parameter("xd", [C, N], FP32R, isOutput=False),
        "wqt": nc.declare_dram_parameter("wqt", [C, C8], FP32R, isOutput=False),
        "wkt": nc.declare_dram_parameter("wkt", [C, C8], FP32R, isOutput=False),
        "wvt": nc.declare_dram_parameter("wvt", [C, C], FP32R, isOutput=False),
        "bq": nc.declare_dram_parameter("bq", [C8, 1], FP32, isOutput=False),
        "bk": nc.declare_dram_parameter("bk", [C8, 1], FP32, isOutput=False),
        "bvb": nc.declare_dram_parameter("bvb", [P, C], FP32, isOutput=False),
        "gam": nc.declare_dram_parameter("gam", [P, 1], FP32, isOutput=False),
        "ident": nc.declare_dram_parameter("ident", [P, P], FP32, isOutput=False),
        "y": nc.declare_dram_parameter("y", [C, NH], FP32, isOutput=True),
    }
    with tile.TileContext(nc) as tc:
        with (
            tc.tile_pool(name="const", bufs=1) as cpool,
            tc.tile_pool(name="io", bufs=1) as iopool,
            tc.tile_pool(name="qkv", bufs=1) as qkvpool,
            tc.tile_pool(name="expp", bufs=2) as epool,
            tc.tile_pool(name="work", bufs=4) as wpool,
        ):
            pools = {
                "const": cpool,
                "io": iopool,
                "qkv": qkvpool,
                "expp": epool,
                "work": wpool,
            }
            if loop_reps == 1:
                emit_body(nc, tc, t, pools)
            else:
                with tc.For_i(0, loop_reps, 1):
                    emit_body(nc, tc, t, pools)
    nc.compile()
    return nc


def make_in_maps(x_ccd, x_dem, Wq, bq, Wk, bk, Wv, bv, gamma):
    xc_all = np.asarray(x_ccd, dtype=np.float32).reshape(B, C, N)
    xd_all = np.asarray(x_dem, dtype=np.float32).reshape(B, C, N)
    shared = {
        "wqt": np.ascontiguousarray(np.asarray(Wq, np.float32).T),
        "wkt": np.ascontiguousarray(np.asarray(Wk, np.float32).T),
        "wvt": np.ascontiguousarray(np.asarray(Wv, np.float32).T),
        "bq": np.asarray(bq, np.float32).reshape(C8, 1),
        "bk": np.asarray(bk, np.float32).reshape(C8, 1),
        "bvb": np.ascontiguousarray(
            np.broadcast_to(np.asarray(bv, np.float32), (P, C))
        ),
        "gam": np.ascontiguousarray(
            np.broadcast_to(np.asarray(gamma, np.float32).reshape(1, 1), (P, 1))
        ),
        "ident": np.eye(P, dtype=np.float32),
    }
    in_maps = []
    for core in range(N_CORES):
        b, h = divmod(core, 2)
        m = dict(shared)
        m["xc"] = np.ascontiguousarray(xc_all[b, :, h * NH : (h + 1) * NH])
        m["xd"] = np.ascontiguousarray(xd_all[b])
        in_maps.append(m)
    return in_maps


_NC_CACHE = {}


def get_nc(loop_reps=1):
    if loop_reps not in _NC_CACHE:
        _NC_CACHE[loop_reps] = build_nc(loop_reps)
    return _NC_CACHE[loop_reps]


def kernel(**inputs):
    in_maps = make_in_maps(
        inputs["x_ccd"],
        inputs["x_dem"],
        inputs["Wq"],
        inputs["bq"],
        inputs["Wk"],
        inputs["bk"],
        inputs["Wv"],
        inputs["bv"],
        inputs["gamma"],
    )
    nc = get_nc()
    res = run_bass_kernel_spmd(nc, in_maps, list(range(N_CORES)))
    y = np.empty((B, C, N), np.float32)
    for core in range(N_CORES):
        b, h = divmod(core, 2)
        y[b, :, h * NH : (h + 1) * NH] = res.results[core]["y"]
    return y.reshape(B, C, H, W)


# revision 7
# speedup vs baseline: 1.1717x; 1.1717x over previous
"""Trainium2 Bass kernel for a SAGAN-style self-attention block.

Reference computation (per batch b):
    xc = x_ccd[b] reshaped [C, N]; xd = x_dem[b] reshaped [C, N]
    q  = (Wq @ xc).T + bq          # [N, 32]
    k  = Wk @ xd + bk              # [32, N]
    e  = q @ k                     # [N, N]
    a  = softmax(e, axis=-1)
    v  = Wv @ xd + bv              # [C, N]
    y  = gamma * (v @ a.T) + x_ccd[b]

Sharding: 8 cores = 4 batches x 2 query-row halves. Each core computes the
full k/v projections for its batch and a 2048-row slice of the attention
output. No collectives needed.

Per-core layout (all "transposed" so softmax's reduction lands on the PE
via an appended ones-column, avoiding partition-axis reductions):
    qt   [32, 2048]   (o on partitions)
    k    [32, 4096]
    vt   [128, 32, 257] bf16  (m on partitions, 32 m-chunks; col 256 = 1.0)
    eT   [m 128, n 512] tiles = k_chunk.T @ qt  -> exp on ACT -> bf16
    outU [n 128, 257]  = sum_m expT_chunk.T @ vt_chunk   (col 256 = denom)
    y    = transpose(outU[:, :256] * gamma/denom) + xc
"""

import numpy as np

import concourse.bacc as bacc
import concourse.mybir as mybir
import concourse.tile as tile
from concourse import bass
from concourse.bass_utils import run_bass_kernel_spmd

B, C, H, W = 4, 256, 64, 64
N = H * W  # 4096
NH = N // 2  # 2048 query rows per core
C8 = 32
P = 128
N_CORES = 8

FP32 = mybir.dt.float32
FP32R = mybir.dt.float32r
BF16 = mybir.dt.bfloat16

ts = bass.ts


def emit_body(nc, tc, t, pools):
    """Emit one iteration of the attention computation.

    t: dict of DRAM parameter handles. pools: dict of tile pools.
    """
    cpool = pools["const"]
    iopool = pools["io"]
    qkvpool = pools["qkv"]
    epool = pools["expp"]
    wpool = pools["work"]

    # ---- constants / weights -------------------------------------------------
    wqt = cpool.tile([P, 2, C8], FP32R, tag="wqt")
    nc.sync.dma_start(wqt[:], t["wqt"][:].rearrange("(j p) o -> p j o", p=P))
    wkt = cpool.tile([P, 2, C8], FP32R, tag="wkt")
    nc.sync.dma_start(wkt[:], t["wkt"][:].rearrange("(j p) o -> p j o", p=P))
    wvt = cpool.tile([P, 2, C], FP32R, tag="wvt")
    nc.sync.dma_start(wvt[:], t["wvt"][:].rearrange("(j p) o -> p j o", p=P))
    bq = cpool.tile([C8, 1], FP32, tag="bq")
    nc.sync.dma_start(bq[:], t["bq"][:])
    bk = cpool.tile([C8, 1], FP32, tag="bk")
    nc.sync.dma_start(bk[:], t["bk"][:])
    bvb = cpool.tile([P, C], FP32, tag="bvb")
    nc.sync.dma_start(bvb[:], t["bvb"][:])
    gam = cpool.tile([P, 1], FP32, tag="gam")
    nc.sync.dma_start(gam[:], t["gam"][:])
    ident = cpool.tile([P, P], FP32, tag="ident")
    nc.sync.dma_start(ident[:], t["ident"][:])

    # ---- activations (split DMAs so they spread across queues) --------------
    xd = iopool.tile([P, 2, N], FP32R, tag="xd")
    xd_r = t["xd"][:].rearrange("(j p) n -> p j n", p=P)
    for j in range(2):
        for s in range(4):
            nc.sync.dma_start(xd[:, j, ts(s, 1024)], xd_r[:, j, ts(s, 1024)])
    xc = iopool.tile([P, 2, NH], FP32, tag="xc")
    xc_r = t["xc"][:].rearrange("(j p) n -> p j n", p=P)
    for j in range(2):
        for s in range(2):
            nc.sync.dma_start(xc[:, j, ts(s, 1024)], xc_r[:, j, ts(s, 1024)])
    y_sb = iopool.tile([P, 2, NH], FP32, tag="y")

    # rounded copy of xc for the fp32r q-projection (xc itself stays exact
    # fp32 for the residual add)
    xcr = qkvpool.tile([P, 2, NH], FP32R, tag="xcr")
    for j in range(2):
        nc.vector.tensor_copy(xcr[:, j, :], xc[:, j, :])

    qt = qkvpool.tile([C8, NH], FP32R, tag="qt")
    k_sb = qkvpool.tile([C8, N], FP32R, tag="k")
    vt = qkvpool.tile([P, 32, C + 1], BF16, tag="vt")

    # ---- projections ---------------------------------------------------------
    with (
        tc.tile_pool(name="ps_p", bufs=2, space="PSUM") as ps_p,
        tc.tile_pool(name="ps_v", bufs=2, space="PSUM") as ps_v,
    ):
        for j in range(N // 512):  # k = Wk @ xd + bk
            pk = ps_p.tile([C8, 512], FP32, tag="pqk")
            nc.tensor.matmul(
                pk[:],
                wkt[:, 0, :],
                xd[:, 0, ts(j, 512)],
                start=True,
                stop=False,
            )
            nc.tensor.matmul(
                pk[:],
                wkt[:, 1, :],
                xd[:, 1, ts(j, 512)],
                start=False,
                stop=True,
            )
            nc.vector.tensor_scalar_add(k_sb[:, ts(j, 512)], pk[:], bk[:])
        for j in range(NH // 512):  # qt = Wq @ xc + bq
            pq = ps_p.tile([C8, 512], FP32, tag="pqk")
            nc.tensor.matmul(
                pq[:],
                wqt[:, 0, :],
                xcr[:, 0, ts(j, 512)],
                start=True,
                stop=False,
            )
            nc.tensor.matmul(
                pq[:],
                wqt[:, 1, :],
                xcr[:, 1, ts(j, 512)],
                start=False,
                stop=True,
            )
            nc.vector.tensor_scalar_add(qt[:, ts(j, 512)], pq[:], bq[:])
        for mi in range(32):  # vt = (Wv @ xd + bv).T, ones col appended
            pv = ps_v.tile([P, C], FP32, tag="pv")
            nc.tensor.matmul(
                pv[:],
                xd[:, 0, ts(mi, 128)],
                wvt[:, 0, :],
                start=True,
                stop=False,
            )
            nc.tensor.matmul(
                pv[:],
                xd[:, 1, ts(mi, 128)],
                wvt[:, 1, :],
                start=False,
                stop=True,
            )
            nc.vector.tensor_add(vt[:, mi, 0:C], pv[:], bvb[:])
            nc.vector.memset(vt[:, mi, C : C + 1], 1.0)

    # ---- attention -----------------------------------------------------------
    # Software-pipelined: energy(mi)/exp(mi) are emitted one step ahead of
    # the outU accumulation matmuls for mi-1 so PE never stalls on ACT.
    with (
        tc.tile_pool(name="ps_e", bufs=2, space="PSUM") as ps_e,
        tc.tile_pool(name="ps_u", bufs=1, space="PSUM") as ps_u,
        tc.tile_pool(name="ps_t", bufs=2, space="PSUM") as ps_t,
    ):
        for nch in range(NH // 512):
            ex = epool.tile([P, 32, 512], BF16, tag="expT")
            pus = [
                ps_u.tile([P, C + 1], FP32, tag=f"outu{ns}", name=f"pu{ns}_{nch}")
                for ns in range(4)
            ]
            for step in range(33):
                mi = step
                if mi < 32:
                    pe = ps_e.tile([P, 512], FP32, tag="en")
                    nc.tensor.matmul(
                        pe[:],
                        k_sb[:, ts(mi, 128)],
                        qt[:, ts(nch, 512)],
                        start=True,
                        stop=True,
                    )
                    nc.scalar.activation(
                        ex[:, mi, :], pe[:], mybir.ActivationFunctionType.Exp
                    )
                mj = step - 1
                if mj >= 0:
                    for ns in range(4):
                        nc.tensor.matmul(
                            pus[ns][:],
                            ex[:, mj, ts(ns, 128)],
                            vt[:, mj, :],
                            start=(mj == 0),
                            stop=(mj == 31),
                        )
            for ns in range(4):
                pu = pus[ns]
                recip = wpool.tile([P, 1], FP32, tag="recip")
                nc.vector.reciprocal(recip[:], pu[:, C : C + 1])
                scale = wpool.tile([P, 1], FP32, tag="scale")
                nc.vector.tensor_mul(scale[:], recip[:], gam[:])
                norm = wpool.tile([P, C], FP32, tag="norm")
                nc.vector.tensor_scalar_mul(norm[:], pu[:, 0:C], scale[:])
                ng = nch * 4 + ns
                for oc in range(2):
                    pt = ps_t.tile([P, P], FP32, tag="tr")
                    nc.tensor.transpose(pt[:], norm[:, ts(oc, 128)], ident[:])
                    nc.vector.tensor_add(
                        y_sb[:, oc, ts(ng, 128)], pt[:], xc[:, oc, ts(ng, 128)]
                    )

    # ---- store ---------------------------------------------------------------
    y_r = t["y"][:].rearrange("(j p) n -> p j n", p=P)
    for j in range(2):
        for s in range(2):
            nc.sync.dma_start(y_r[:, j, ts(s, 1024)], y_sb[:, j, ts(s, 1024)])


def build_nc(loop_reps=1):
    nc = bacc.Bacc("TRN2", target_bir_lowering=False, debug=False, num_devices=N_CORES)
    t = {
        "xc": nc.declare_dram_parameter("xc", [C, NH], FP32, isOutput=False),
        "xd": nc.declare_dram_parameter("xd", [C, N], FP32R, isOutput=False),
        "wqt": nc.declare_dram_parameter("wqt", [C, C8], FP32R, isOutput=False),
        "wkt": nc.declare_dram_parameter("wkt", [C, C8], FP32R, isOutput=False),
        "wvt": nc.declare_dram_parameter("wvt", [C, C], FP32R, isOutput=False),
        "bq": nc.declare_dram_parameter("bq", [C8, 1], FP32, isOutput=False),
        "bk": nc.declare_dram_parameter("bk", [C8, 1], FP32, isOutput=False),
        "bvb": nc.declare_dram_parameter("bvb", [P, C], FP32, isOutput=False),
        "gam": nc.declare_dram_parameter("gam", [P, 1], FP32, isOutput=False),
        "ident": nc.declare_dram_parameter("ident", [P, P], FP32, isOutput=False),
        "y": nc.declare_dram_parameter("y", [C, NH], FP32, isOutput=True),
    }
    with tile.TileContext(nc) as tc:
        with (
            tc.tile_pool(name="const", bufs=1) as cpool,
            tc.tile_pool(name="io", bufs=1) as iopool,
            tc.tile_pool(name="qkv", bufs=1) as qkvpool,
            tc.tile_pool(name="expp", bufs=2) as epool,
            tc.tile_pool(name="work", bufs=4) as wpool,
        ):
            pools = {
                "const": cpool,
                "io": iopool,
                "qkv": qkvpool,
                "expp": epool,
                "work": wpool,
            }
            if loop_reps == 1:
                emit_body(nc, tc, t, pools)
            else:
                with tc.For_i(0, loop_reps, 1):
                    emit_body(nc, tc, t, pools)
    nc.compile()
    return nc


def make_in_maps(x_ccd, x_dem, Wq, bq, Wk, bk, Wv, bv, gamma):
    xc_all = np.asarray(x_ccd, dtype=np.float32).reshape(B, C, N)
    xd_all = np.asarray(x_dem, dtype=np.float32).reshape(B, C, N)
    shared = {
        "wqt": np.ascontiguousarray(np.asarray(Wq, np.float32).T),
        "wkt": np.ascontiguousarray(np.asarray(Wk, np.float32).T),
        "wvt": np.ascontiguousarray(np.asarray(Wv, np.float32).T),
        "bq": np.asarray(bq, np.float32).reshape(C8, 1),
        "bk": np.asarray(bk, np.float32).reshape(C8, 1),
        "bvb": np.ascontiguousarray(
            np.broadcast_to(np.asarray(bv, np.float32), (P, C))
        ),
        "gam": np.ascontiguousarray(
            np.broadcast_to(np.asarray(gamma, np.float32).reshape(1, 1), (P, 1))
        ),
        "ident": np.eye(P, dtype=np.float32),
    }
    in_maps = []
    for core in range(N_CORES):
        b, h = divmod(core, 2)
        m = dict(shared)
        m["xc"] = np.ascontiguousarray(xc_all[b, :, h * NH : (h + 1) * NH])
        m["xd"] = np.ascontiguousarray(xd_all[b])
        in_maps.append(m)
    return in_maps


_NC_CACHE = {}


def get_nc(loop_reps=1):
    if loop_reps not in _NC_CACHE:
        _NC_CACHE[loop_reps] = build_nc(loop_reps)
    return _NC_CACHE[loop_reps]


def kernel(**inputs):
    in_maps = make_in_maps(
        inputs["x_ccd"],
        inputs["x_dem"],
        inputs["Wq"],
        inputs["bq"],
        inputs["Wk"],
        inputs["bk"],
        inputs["Wv"],
        inputs["bv"],
        inputs["gamma"],
    )
    nc = get_nc()
    res = run_bass_kernel_spmd(nc, in_maps, list(range(N_CORES)))
    y = np.empty((B, C, N), np.float32)
    for core in range(N_CORES):
        b, h = divmod(core, 2)
        y[b, :, h * NH : (h + 1) * NH] = res.results[core]["y"]
    return y.reshape(B, C, H, W)


# revision 8
# speedup vs baseline: 1.2734x; 1.0867x over previous
"""Trainium2 Bass kernel for a SAGAN-style self-attention block.

Reference computation (per batch b):
    xc = x_ccd[b] reshaped [C, N]; xd = x_dem[b] reshaped [C, N]
    q  = (Wq @ xc).T + bq          # [N, 32]
    k  = Wk @ xd + bk              # [32, N]
    e  = q @ k                     # [N, N]
    a  = softmax(e, axis=-1)
    v  = Wv @ xd + bv              # [C, N]
    y  = gamma * (v @ a.T) + x_ccd[b]

Sharding: 8 cores = 4 batches x 2 query-row halves. Each core computes the
full k/v projections for its batch and a 2048-row slice of the attention
output. No collectives needed.

Per-core layout (all "transposed" so softmax's reduction lands on the PE
via an appended ones-column, avoiding partition-axis reductions):
    qt   [32, 2048]   (o on partitions)
    k    [32, 4096]
    vt   [128, 32, 257] bf16  (m on partitions, 32 m-chunks; col 256 = 1.0)
    eT   [m 128, n 512] tiles = k_chunk.T @ qt  -> exp on ACT -> bf16
    outU [n 128, 257]  = sum_m expT_chunk.T @ vt_chunk   (col 256 = denom)
    y    = transpose(outU[:, :256] * gamma/denom) + xc
"""

import numpy as np

import concourse.bacc as bacc
import concourse.mybir as mybir
import concourse.tile as tile
from concourse import bass
from concourse.bass_utils import run_bass_kernel_spmd

B, C, H, W = 4, 256, 64, 64
N = H * W  # 4096
NH = N // 2  # 2048 query rows per core
C8 = 32
P = 128
N_CORES = 8

FP32 = mybir.dt.float32
FP32R = mybir.dt.float32r
BF16 = mybir.dt.bfloat16

# dtype of the q/k tensors feeding the energy matmul (fp32r default;
# KERNEL_QK_BF16=1 switches to bf16 for experimentation)
import os as _os
QK_DT = BF16 if _os.environ.get("KERNEL_QK_BF16") == "1" else FP32R

ts = bass.ts


def emit_body(nc, tc, t, pools):
    """Emit one iteration of the attention computation.

    t: dict of DRAM parameter handles. pools: dict of tile pools.
    """
    cpool = pools["const"]
    iopool = pools["io"]
    qkvpool = pools["qkv"]
    epool = pools["expp"]
    wpool = pools["work"]

    # ---- constants / weights -------------------------------------------------
    wqt = cpool.tile([P, 2, C8], FP32R, tag="wqt")
    nc.sync.dma_start(wqt[:], t["wqt"][:].rearrange("(j p) o -> p j o", p=P))
    wkt = cpool.tile([P, 2, C8], FP32R, tag="wkt")
    nc.sync.dma_start(wkt[:], t["wkt"][:].rearrange("(j p) o -> p j o", p=P))
    wvt = cpool.tile([P, 2, C], FP32R, tag="wvt")
    nc.sync.dma_start(wvt[:], t["wvt"][:].rearrange("(j p) o -> p j o", p=P))
    bq = cpool.tile([C8, 1], FP32, tag="bq")
    nc.sync.dma_start(bq[:], t["bq"][:])
    bk = cpool.tile([C8, 1], FP32, tag="bk")
    nc.sync.dma_start(bk[:], t["bk"][:])
    bvb = cpool.tile([P, C], FP32, tag="bvb")
    nc.sync.dma_start(bvb[:], t["bvb"][:])
    gam = cpool.tile([P, 1], FP32, tag="gam")
    nc.sync.dma_start(gam[:], t["gam"][:])
    ident = cpool.tile([P, P], FP32, tag="ident")
    nc.sync.dma_start(ident[:], t["ident"][:])

    # ---- activations (split DMAs so they spread across queues) --------------
    xd = iopool.tile([P, 2, N], FP32R, tag="xd")
    xd_r = t["xd"][:].rearrange("(j p) n -> p j n", p=P)
    for j in range(2):
        for s in range(4):
            nc.sync.dma_start(xd[:, j, ts(s, 1024)], xd_r[:, j, ts(s, 1024)])
    xc = iopool.tile([P, 2, NH], FP32, tag="xc")
    xc_r = t["xc"][:].rearrange("(j p) n -> p j n", p=P)
    for j in range(2):
        for s in range(2):
            nc.sync.dma_start(xc[:, j, ts(s, 1024)], xc_r[:, j, ts(s, 1024)])
    y_sb = iopool.tile([P, 2, NH], FP32, tag="y")

    # rounded copy of xc for the fp32r q-projection (xc itself stays exact
    # fp32 for the residual add)
    xcr = qkvpool.tile([P, 2, NH], FP32R, tag="xcr")
    for j in range(2):
        nc.vector.tensor_copy(xcr[:, j, :], xc[:, j, :])

    qt = qkvpool.tile([C8, NH], QK_DT, tag="qt")
    k_sb = qkvpool.tile([C8, N], QK_DT, tag="k")
    vt = qkvpool.tile([P, 32, C + 1], BF16, tag="vt")

    # ---- projections ---------------------------------------------------------
    with (
        tc.tile_pool(name="ps_p", bufs=2, space="PSUM") as ps_p,
        tc.tile_pool(name="ps_v", bufs=2, space="PSUM") as ps_v,
    ):
        for j in range(N // 512):  # k = Wk @ xd + bk
            pk = ps_p.tile([C8, 512], FP32, tag="pqk")
            nc.tensor.matmul(
                pk[:],
                wkt[:, 0, :],
                xd[:, 0, ts(j, 512)],
                start=True,
                stop=False,
            )
            nc.tensor.matmul(
                pk[:],
                wkt[:, 1, :],
                xd[:, 1, ts(j, 512)],
                start=False,
                stop=True,
            )
            nc.vector.tensor_scalar_add(k_sb[:, ts(j, 512)], pk[:], bk[:])
        for j in range(NH // 512):  # qt = Wq @ xc + bq
            pq = ps_p.tile([C8, 512], FP32, tag="pqk")
            nc.tensor.matmul(
                pq[:],
                wqt[:, 0, :],
                xcr[:, 0, ts(j, 512)],
                start=True,
                stop=False,
            )
            nc.tensor.matmul(
                pq[:],
                wqt[:, 1, :],
                xcr[:, 1, ts(j, 512)],
                start=False,
                stop=True,
            )
            nc.vector.tensor_scalar_add(qt[:, ts(j, 512)], pq[:], bq[:])
        for mi in range(32):  # vt = (Wv @ xd + bv).T, ones col appended
            pv = ps_v.tile([P, C], FP32, tag="pv")
            nc.tensor.matmul(
                pv[:],
                xd[:, 0, ts(mi, 128)],
                wvt[:, 0, :],
                start=True,
                stop=False,
            )
            nc.tensor.matmul(
                pv[:],
                xd[:, 1, ts(mi, 128)],
                wvt[:, 1, :],
                start=False,
                stop=True,
            )
            nc.vector.tensor_add(vt[:, mi, 0:C], pv[:], bvb[:])
            nc.vector.memset(vt[:, mi, C : C + 1], 1.0)

    # ---- attention -----------------------------------------------------------
    # Software-pipelined: energy(mi)/exp(mi) are emitted one step ahead of
    # the outU accumulation matmuls for mi-1 so PE never stalls on ACT.
    with (
        tc.tile_pool(name="ps_e", bufs=2, space="PSUM") as ps_e,
        tc.tile_pool(name="ps_u", bufs=1, space="PSUM") as ps_u,
        tc.tile_pool(name="ps_t", bufs=2, space="PSUM") as ps_t,
    ):
        for nch in range(NH // 512):
            ex = epool.tile([P, 32, 512], BF16, tag="expT")
            pus = [
                ps_u.tile([P, C + 1], FP32, tag=f"outu{ns}", name=f"pu{ns}_{nch}")
                for ns in range(4)
            ]
            for step in range(33):
                mi = step
                if mi < 32:
                    pe = ps_e.tile([P, 512], FP32, tag="en")
                    nc.tensor.matmul(
                        pe[:],
                        k_sb[:, ts(mi, 128)],
                        qt[:, ts(nch, 512)],
                        start=True,
                        stop=True,
                    )
                    nc.scalar.activation(
                        ex[:, mi, :], pe[:], mybir.ActivationFunctionType.Exp
                    )
                mj = step - 1
                if mj >= 0:
                    for ns in range(4):
                        nc.tensor.matmul(
                            pus[ns][:],
                            ex[:, mj, ts(ns, 128)],
                            vt[:, mj, :],
                            start=(mj == 0),
                            stop=(mj == 31),
                        )
            for ns in range(4):
                pu = pus[ns]
                recip = wpool.tile([P, 1], FP32, tag="recip")
                nc.vector.reciprocal(recip[:], pu[:, C : C + 1])
                scale = wpool.tile([P, 1], FP32, tag="scale")
                nc.vector.tensor_mul(scale[:], recip[:], gam[:])
                norm = wpool.tile([P, C], FP32, tag="norm")
                nc.vector.tensor_scalar_mul(norm[:], pu[:, 0:C], scale[:])
                ng = nch * 4 + ns
                for oc in range(2):
                    pt = ps_t.tile([P, P], FP32, tag="tr")
                    nc.tensor.transpose(pt[:], norm[:, ts(oc, 128)], ident[:])
                    nc.vector.tensor_add(
                        y_sb[:, oc, ts(ng, 128)], pt[:], xc[:, oc, ts(ng, 128)]
                    )

    # ---- store ---------------------------------------------------------------
    y_r = t["y"][:].rearrange("(j p) n -> p j n", p=P)
    for j in range(2):
        for s in range(2):
            nc.sync.dma_start(y_r[:, j, ts(s, 1024)], y_sb[:, j, ts(s, 1024)])


def build_nc(loop_reps=1):
    nc = bacc.Bacc("TRN2", target_bir_lowering=False, debug=False, num_devices=N_CORES)
    t = {
        "xc": nc.declare_dram_parameter("xc", [C, NH], FP32, isOutput=False),
        "xd": nc.declare_dram_parameter("xd", [C, N], FP32R, isOutput=False),
        "wqt": nc.declare_dram_parameter("wqt", [C, C8], FP32R, isOutput=False),
        "wkt": nc.declare_dram_parameter("wkt", [C, C8], FP32R, isOutput=False),
        "wvt": nc.declare_dram_parameter("wvt", [C, C], FP32R, isOutput=False),
        "bq": nc.declare_dram_parameter("bq", [C8, 1], FP32, isOutput=False),
        "bk": nc.declare_dram_parameter("bk", [C8, 1], FP32, isOutput=False),
        "bvb": nc.declare_dram_parameter("bvb", [P, C], FP32, isOutput=False),
        "gam": nc.declare_dram_parameter("gam", [P, 1], FP32, isOutput=False),
        "ident": nc.declare_dram_parameter("ident", [P, P], FP32, isOutput=False),
        "y": nc.declare_dram_parameter("y", [C, NH], FP32, isOutput=True),
    }
    with tile.TileContext(nc) as tc:
        with (
            tc.tile_pool(name="const", bufs=1) as cpool,
            tc.tile_pool(name="io", bufs=1) as iopool,
            tc.tile_pool(name="qkv", bufs=1) as qkvpool,
            tc.tile_pool(name="expp", bufs=2) as epool,
            tc.tile_pool(name="work", bufs=4) as wpool,
        ):
            pools = {
                "const": cpool,
                "io": iopool,
                "qkv": qkvpool,
                "expp": epool,
                "work": wpool,
            }
            if loop_reps == 1:
                emit_body(nc, tc, t, pools)
            else:
                with tc.For_i(0, loop_reps, 1):
                    emit_body(nc, tc, t, pools)
    nc.compile()
    return nc


def make_in_maps(x_ccd, x_dem, Wq, bq, Wk, bk, Wv, bv, gamma):
    xc_all = np.asarray(x_ccd, dtype=np.float32).reshape(B, C, N)
    xd_all = np.asarray(x_dem, dtype=np.float32).reshape(B, C, N)
    shared = {
        "wqt": np.ascontiguousarray(np.asarray(Wq, np.float32).T),
        "wkt": np.ascontiguousarray(np.asarray(Wk, np.float32).T),
        "wvt": np.ascontiguousarray(np.asarray(Wv, np.float32).T),
        "bq": np.asarray(bq, np.float32).reshape(C8, 1),
        "bk": np.asarray(bk, np.float32).reshape(C8, 1),
        "bvb": np.ascontiguousarray(
            np.broadcast_to(np.asarray(bv, np.float32), (P, C))
        ),
        "gam": np.ascontiguousarray(
            np.broadcast_to(np.asarray(gamma, np.float32).reshape(1, 1), (P, 1))
        ),
        "ident": np.eye(P, dtype=np.float32),
    }
    in_maps = []
    for core in range(N_CORES):
        b, h = divmod(core, 2)
        m = dict(shared)
        m["xc"] = np.ascontiguousarray(xc_all[b, :, h * NH : (h + 1) * NH])
        m["xd"] = np.ascontiguousarray(xd_all[b])
        in_maps.append(m)
    return in_maps


_NC_CACHE = {}


def get_nc(loop_reps=1):
    if loop_reps not in _NC_CACHE:
        _NC_CACHE[loop_reps] = build_nc(loop_reps)
    return _NC_CACHE[loop_reps]


def kernel(**inputs):
    in_maps = make_in_maps(
        inputs["x_ccd"],
        inputs["x_dem"],
        inputs["Wq"],
        inputs["bq"],
        inputs["Wk"],
        inputs["bk"],
        inputs["Wv"],
        inputs["bv"],
        inputs["gamma"],
    )
    nc = get_nc()
    res = run_bass_kernel_spmd(nc, in_maps, list(range(N_CORES)))
    y = np.empty((B, C, N), np.float32)
    for core in range(N_CORES):
        b, h = divmod(core, 2)
        y[b, :, h * NH : (h + 1) * NH] = res.results[core]["y"]
    return y.reshape(B, C, H, W)
